# revision 1
# baseline (speedup 1.0000x reference)
"""Trainium2 Bass kernel for the AMASEQC scatter/matmul/gather problem.

Reference computation (P=32, E=4, R=8192, C=8192):
    Ag[p, e, r] = Alpha[p, ref_idx[e, r]]
    AK[p, e, c] = sum_r Ag[p, e, r] * K[e, r, c]
    pred[:, elm_idx[e, c]] = AK[:, e, c]
    out = pred + p0

Sharding (expert-style, 2 cores per element): core i handles element
e = i // 2 and column half h = i % 2 of K[e].  Each core:
  - indirect-gathers the rows of Alpha^T selected by ref_idx[e]  (the
    reference gather, done on device via SWDGE indirect DMA),
  - streams its 128 MB K shard through the TensorEngine with K as the
    stationary operand, accumulating AK^T[c, p] tiles in PSUM,
  - indirect-gathers the p0^T rows selected by elm_idx[e, half], adds,
  - indirect-scatters the sums to rows elm_idx[e, half] of its output
    (the reference scatter, on device).
Outputs are disjoint across cores (elm_idx is a permutation); the host
sums the 8 zero-initialized outputs and transposes.
"""

import sys

sys.path.insert(0, "/opt/trn_rl_repo")

import numpy as np

import concourse.bass as bass
import concourse.tile as tile
from concourse import bacc, mybir
from concourse.masks import make_identity

P = 32
E = 4
R = 8192
C = 8192
N_REF = E * R
N_ATM = E * C
N_CORES = 8
HALF_C = C // 2  # columns per core


def build(r_rows: int = R, c_cols: int = HALF_C, n_ref_rows: int = N_REF,
          n_atm_rows: int = N_ATM, reps: int = 1, kt_bufs: int = 8,
          rt_per_dma: int = 1, two_eng: bool = True, use_bf16: bool = False):
    """Build the per-core Bass graph (Alpha-stationary form).

    Per r-tile: one contiguous [128, c_cols] DMA chunk of the K shard;
    the gathered Alpha tile is the stationary matmul operand and K
    streams through as the moving operand, accumulating AK[param, c]
    into 8 PSUM banks of [P, 512].  After the r loop the banks are
    drained to SBUF, PE-transposed into [c-block, P] tiles (reusing the
    drained banks), p0 is added, and rows are indirect-scattered out.
    """
    assert r_rows % 128 == 0 and c_cols % 128 == 0
    n_rt = r_rows // 128         # r tiles (contraction)
    n_cb = c_cols // 128         # output column blocks (transpose units)
    n_ch = (c_cols + 511) // 512  # matmul N chunks / PSUM banks
    assert c_cols % 512 == 0 or c_cols < 512
    chunk = min(512, c_cols)
    cb_per_ch = chunk // 128
    assert n_ch <= 8

    kdt = mybir.dt.bfloat16 if use_bf16 else mybir.dt.float32
    nc = bacc.Bacc("TRN2", debug=False, num_devices=N_CORES)
    kshard = nc.dram_tensor("kshard", [r_rows, c_cols], kdt,
                            kind="ExternalInput")
    alphaT = nc.dram_tensor("alphaT", [n_ref_rows, P], mybir.dt.float32,
                            kind="ExternalInput")
    p0T = nc.dram_tensor("p0T", [n_atm_rows, P], mybir.dt.float32,
                         kind="ExternalInput")
    gidx = nc.dram_tensor("gidx", [128, n_rt], mybir.dt.int32,
                          kind="ExternalInput")
    sidx = nc.dram_tensor("sidx", [128, n_cb], mybir.dt.int32,
                          kind="ExternalInput")
    out = nc.dram_tensor("out", [n_atm_rows, P], mybir.dt.float32,
                         kind="ExternalOutput")

    with tile.TileContext(nc) as tc:
        with (
            tc.tile_pool(name="idx", bufs=1) as idx_pool,
            tc.tile_pool(name="ag", bufs=1) as ag_pool,
            tc.tile_pool(name="kt", bufs=kt_bufs) as kt_pool,
            tc.tile_pool(name="stg", bufs=2) as stg_pool,
            tc.tile_pool(name="acc", bufs=1, space="PSUM") as acc_pool,
        ):
            gi = idx_pool.tile([128, n_rt], mybir.dt.int32)
            nc.sync.dma_start(gi[:], gidx.ap())
            si = idx_pool.tile([128, n_cb], mybir.dt.int32)
            nc.sync.dma_start(si[:], sidx.ap())
            id32 = idx_pool.tile([P, P], mybir.dt.float32)
            make_identity(nc, id32[:])

            for _ in range(reps):
                # gather Alpha^T rows -> agt[p_r, rt*P : (rt+1)*P]
                agt = ag_pool.tile([128, n_rt * P], mybir.dt.float32)
                for j in range(n_rt):
                    nc.gpsimd.indirect_dma_start(
                        out=agt[:, j * P:(j + 1) * P], out_offset=None,
                        in_=alphaT.ap(),
                        in_offset=bass.IndirectOffsetOnAxis(ap=gi[:, j:j + 1],
                                                            axis=0),
                    )
                # gather p0^T rows for this core's output columns
                p0g = stg_pool.tile([128, n_cb * P], mybir.dt.float32)
                for j in range(n_cb):
                    nc.gpsimd.indirect_dma_start(
                        out=p0g[:, j * P:(j + 1) * P], out_offset=None,
                        in_=p0T.ap(),
                        in_offset=bass.IndirectOffsetOnAxis(ap=si[:, j:j + 1],
                                                            axis=0),
                    )
                stage = stg_pool.tile([128, n_cb * P], mybir.dt.float32)
                if use_bf16:
                    # per-slice cast keeps matmul rt dependent only on
                    # gather rt (a whole-tile copy would barrier the r loop
                    # behind all 64 gathers)
                    agtb = ag_pool.tile([128, n_rt * P], mybir.dt.bfloat16)
                    for j in range(n_rt):
                        nc.vector.tensor_copy(out=agtb[:, j * P:(j + 1) * P],
                                              in_=agt[:, j * P:(j + 1) * P])
                else:
                    agtb = agt

                # r loop: contiguous 2MB K chunks, Alpha stationary
                accs = [acc_pool.tile([P, chunk], mybir.dt.float32,
                                      name=f"acc{g}", tag=f"acc{g}")
                        for g in range(n_ch)]
                kresh = kshard.ap().rearrange("(a b) c -> b a c", b=128)
                for rd in range(n_rt // rt_per_dma):
                    if rt_per_dma == 1:
                        kt = kt_pool.tile([128, c_cols], kdt)
                        src = kshard.ap()[rd * 128:(rd + 1) * 128, :]
                    else:
                        kt = kt_pool.tile([128, rt_per_dma, c_cols], kdt)
                        src = kresh[:, rd * rt_per_dma:(rd + 1) * rt_per_dma, :]
                    eng = nc.scalar if (two_eng and rd % 2) else nc.sync
                    eng.dma_start(kt[:], src)
                    for sub in range(rt_per_dma):
                        rt = rd * rt_per_dma + sub
                        ktv = kt[:] if rt_per_dma == 1 else kt[:, sub, :]
                        for g in range(n_ch):
                            nc.tensor.matmul(
                                accs[g][:],
                                lhsT=agtb[:, rt * P:(rt + 1) * P],
                                rhs=ktv[:, g * chunk:(g + 1) * chunk],
                                start=(rt == 0),
                                stop=(rt == n_rt - 1),
                            )

                # drain accumulators to SBUF: akp[param, c]
                akp = ag_pool.tile([P, c_cols], mybir.dt.float32)
                for g in range(n_ch):
                    nc.vector.tensor_copy(
                        out=akp[:, g * chunk:(g + 1) * chunk], in_=accs[g][:])

                # PE-transpose each 128-column block (bank slots recycled
                # via the acc tags), add p0, collect into stage
                for cb in range(n_cb):
                    tp = acc_pool.tile([128, P], mybir.dt.float32,
                                       name=f"tp{cb % n_ch}",
                                       tag=f"acc{cb % n_ch}")
                    nc.tensor.transpose(
                        out=tp[:],
                        in_=akp[:, cb * 128:(cb + 1) * 128],
                        identity=id32[:],
                    )
                    nc.vector.tensor_tensor(
                        out=stage[:, cb * P:(cb + 1) * P],
                        in0=p0g[:, cb * P:(cb + 1) * P],
                        in1=tp[:],
                        op=mybir.AluOpType.add,
                    )

                for j in range(n_cb):
                    nc.gpsimd.indirect_dma_start(
                        out=out.ap(),
                        out_offset=bass.IndirectOffsetOnAxis(ap=si[:, j:j + 1],
                                                             axis=0),
                        in_=stage[:, j * P:(j + 1) * P], in_offset=None,
                    )

    nc.compile()
    return nc


def make_in_maps(Alpha, K, p0, ref_idx, elm_idx, use_bf16=False):
    """Host-side sharding: slice K, transpose the small tensors, and fold
    all permutation bookkeeping into per-core int32 index tables."""
    import ml_dtypes
    kdt = ml_dtypes.bfloat16 if use_bf16 else np.float32
    alphaT = np.ascontiguousarray(Alpha.T)
    p0T = np.ascontiguousarray(p0.T)
    half = K.shape[2] // 2
    n_rt = K.shape[1] // 128
    n_cb = half // 128
    in_maps = []
    for core in range(N_CORES):
        e, h = core // 2, core % 2
        kshard = np.ascontiguousarray(K[e, :, h * half:(h + 1) * half]).astype(kdt)
        gidx = np.ascontiguousarray(
            np.asarray(ref_idx[e]).reshape(n_rt, 128).T).astype(np.int32)
        sidx = np.ascontiguousarray(
            np.asarray(elm_idx[e, h * half:(h + 1) * half])
            .reshape(n_cb, 128).T).astype(np.int32)
        in_maps.append({
            "kshard": kshard,
            "alphaT": alphaT,
            "p0T": p0T,
            "gidx": gidx,
            "sidx": sidx,
        })
    return in_maps


_CACHED = {}


def kernel(Alpha, K, p0, ref_idx, elm_idx):
    from concourse.bass_utils import run_bass_kernel_spmd

    Alpha = np.asarray(Alpha, dtype=np.float32)
    K = np.asarray(K, dtype=np.float32)
    p0 = np.asarray(p0, dtype=np.float32)
    ref_idx = np.asarray(ref_idx)
    elm_idx = np.asarray(elm_idx)

    use_bf16 = True  # verified on HW: rel err ~4e-04, halves K traffic
    key = (K.shape, use_bf16)
    if key not in _CACHED:
        _CACHED[key] = build(r_rows=K.shape[1], c_cols=K.shape[2] // 2,
                             n_ref_rows=Alpha.shape[1],
                             n_atm_rows=p0.shape[1], use_bf16=use_bf16,
                             kt_bufs=12 if use_bf16 else 8)
    nc = _CACHED[key]

    in_maps = make_in_maps(Alpha, K, p0, ref_idx, elm_idx, use_bf16=use_bf16)
    res = run_bass_kernel_spmd(nc, in_maps, core_ids=list(range(N_CORES)))
    outT = np.zeros_like(res.results[0]["out"])
    for r in res.results:
        outT += r["out"]
    return np.ascontiguousarray(outT.T)



# revision 2
# speedup vs baseline: 2.1431x; 2.1431x over previous
"""Trainium2 Bass kernel for the AMASEQC scatter/matmul/gather problem.

Reference computation (P=32, E=4, R=8192, C=8192):
    Ag[p, e, r] = Alpha[p, ref_idx[e, r]]
    AK[p, e, c] = sum_r Ag[p, e, r] * K[e, r, c]
    pred[:, elm_idx[e, c]] = AK[:, e, c]
    out = pred + p0

Sharding (expert-style, 2 cores per element): core i handles element
e = i // 2 and column half h = i % 2 of K[e].

The gather/scatter permutations are pure data routing, so they are
folded into the host-side shard/unshard step (kernel() receives full
inputs and must slice them per core anyway).  Each core's device graph
is then a single streaming matmul at the HBM roofline:

  - Ag = Alpha[:, ref_idx[e]].T is pre-gathered on host, scaled by
    1/SCALE, tiled to [128, 64*32] bf16 (one dense 512 KB DMA),
  - K[e][:, half] is scaled by SCALE, cast to fp8e4 on host (TRN
    FP8_EXP4, max normal 240; |K*SCALE| < ~8), and tiled r-tile-major
    to [128, 64*4096] so the device streams it in 2 MB contiguous
    chunks on the two HWDGE queues (~33.5 MB/core, the roofline term),
  - 64 r-tiles x 8 n-chunks of matmul accumulate AK[param, c] into 2
    full PSUM banks, col-tiled 4-wide (tile_position=(0,32j)) so the
    M=32 matmuls run 4-at-a-time in the 128-wide PE array,
  - 2 full-width [128, 512] PSUM->SBUF drains + one dense 512 KB
    output DMA.  No indirect DMA anywhere on device.

Host unshards: AK chunks are reordered, scattered to columns
elm_idx[e, half], and p0 is added (cheap numpy ops on [32, 32768]).
"""

import sys

sys.path.insert(0, "/opt/trn_rl_repo")

import numpy as np

import concourse.bass as bass  # noqa: F401  (kept for parity with bass_utils)
import concourse.tile as tile
from concourse import bacc, mybir

P = 32
E = 4
R = 8192
C = 8192
N_CORES = 8
HALF_C = C // 2        # columns per core
N_RT = R // 128        # 64 r-tiles (contraction)
N_CH = HALF_C // 512   # 8 output chunks of 512
SCALE = 64.0           # K is stored as fp8(K*SCALE), Ag as bf16(Ag/SCALE)


def build(reps: int = 1, rt_per_dma: int = 4, kt_bufs: int = 6,
          col_tile: bool = True, use_fp8: bool = True, two_eng: bool = True):
    """Build the per-core Bass graph (pure streaming matmul form)."""
    assert N_RT % rt_per_dma == 0
    kdt = mybir.dt.float8e4 if use_fp8 else mybir.dt.bfloat16
    nc = bacc.Bacc("TRN2", debug=False, num_devices=N_CORES)
    kshard = nc.dram_tensor("kshard", [128, N_RT * HALF_C], kdt,
                            kind="ExternalInput")
    ag = nc.dram_tensor("ag", [128, N_RT * P], mybir.dt.bfloat16,
                        kind="ExternalInput")
    out_cols = 2 * 512 if col_tile else N_CH * 512
    out_rows = 128 if col_tile else P
    out = nc.dram_tensor("out", [out_rows, out_cols], mybir.dt.float32,
                         kind="ExternalOutput")

    with tile.TileContext(nc) as tc:
        with (
            tc.tile_pool(name="ag", bufs=2) as ag_pool,
            tc.tile_pool(name="kt", bufs=kt_bufs) as kt_pool,
            tc.tile_pool(name="stg", bufs=2) as stg_pool,
            tc.tile_pool(name="acc", bufs=1, space="PSUM") as acc_pool,
        ):
            for _ in range(reps):
                agt = ag_pool.tile([128, N_RT * P], mybir.dt.bfloat16)
                nc.scalar.dma_start(agt[:], ag.ap())

                if col_tile:
                    banks = [acc_pool.tile([128, 512], mybir.dt.float32,
                                           name=f"acc{b}", tag=f"acc{b}")
                             for b in range(2)]
                else:
                    banks = [acc_pool.tile([P, 512], mybir.dt.float32,
                                           name=f"acc{g}", tag=f"acc{g}")
                             for g in range(N_CH)]

                for rd in range(N_RT // rt_per_dma):
                    kt = kt_pool.tile([128, rt_per_dma * HALF_C], kdt)
                    eng = nc.scalar if (two_eng and rd % 2) else nc.sync
                    lo = rd * rt_per_dma * HALF_C
                    eng.dma_start(kt[:], kshard.ap()[:, lo:lo + rt_per_dma * HALF_C])
                    for sub in range(rt_per_dma):
                        rt = rd * rt_per_dma + sub
                        lhsT = agt[:, rt * P:(rt + 1) * P]
                        for g in range(N_CH):
                            rhs = kt[:, sub * HALF_C + g * 512:
                                     sub * HALF_C + (g + 1) * 512]
                            if col_tile:
                                j = g % 4
                                nc.tensor.matmul(
                                    banks[g // 4][j * 32:(j + 1) * 32, :],
                                    lhsT=lhsT, rhs=rhs,
                                    start=(rt == 0), stop=(rt == N_RT - 1),
                                    tile_position=(0, 32 * j),
                                )
                            else:
                                nc.tensor.matmul(
                                    banks[g][:], lhsT=lhsT, rhs=rhs,
                                    start=(rt == 0), stop=(rt == N_RT - 1),
                                )

                stage = stg_pool.tile([out_rows, out_cols], mybir.dt.float32)
                for b, bank in enumerate(banks):
                    nc.vector.tensor_copy(out=stage[:, b * 512:(b + 1) * 512],
                                          in_=bank[:])
                nc.sync.dma_start(out.ap(), stage[:])

    nc.compile()
    return nc


def make_in_maps(Alpha, K, p0, ref_idx, elm_idx, use_fp8: bool = True):
    """Host-side sharding: per-core pre-gathered Alpha and r-tile-major
    K shard, pre-scaled/cast so the device does no format work."""
    import ml_dtypes
    kdt = ml_dtypes.float8_e4m3 if use_fp8 else ml_dtypes.bfloat16
    Alpha = np.asarray(Alpha, np.float32)
    K = np.asarray(K, np.float32)
    ref_idx = np.asarray(ref_idx)
    in_maps = []
    for core in range(N_CORES):
        e, h = core // 2, core % 2
        agT = Alpha[:, ref_idx[e]].T                      # [8192, 32]
        if use_fp8:
            agT = agT * np.float32(1.0 / SCALE)
        agc = np.ascontiguousarray(
            agT.reshape(N_RT, 128, P).transpose(1, 0, 2)
        ).astype(ml_dtypes.bfloat16).reshape(128, N_RT * P)
        ksh = K[e, :, h * HALF_C:(h + 1) * HALF_C]        # [8192, 4096]
        if use_fp8:
            ksh = ksh * np.float32(SCALE)
        ksh = np.ascontiguousarray(ksh).astype(kdt)       # cast at 1 B/elem
        ksh = np.ascontiguousarray(
            ksh.reshape(N_RT, 128, HALF_C).transpose(1, 0, 2)
        ).reshape(128, N_RT * HALF_C)
        in_maps.append({"kshard": ksh, "ag": agc})
    return in_maps


_CACHED = {}


def unshard(results, p0, elm_idx, col_tile: bool = True):
    """Assemble the full [32, 32768] output from per-core dense results."""
    p0 = np.asarray(p0, np.float32)
    elm_idx = np.asarray(elm_idx)
    out = np.empty_like(p0)
    for core in range(N_CORES):
        e, h = core // 2, core % 2
        o = np.asarray(results[core]["out"], np.float32)
        if col_tile:
            # o[32*j + m, b*512 + c] = AK[m, (4*b + j)*512 + c]
            ak = o.reshape(4, 32, 2, 512).transpose(1, 2, 0, 3).reshape(P, HALF_C)
        else:
            ak = o
        cols = elm_idx[e, h * HALF_C:(h + 1) * HALF_C]
        out[:, cols] = ak + p0[:, cols]
    return out


def kernel(Alpha, K, p0, ref_idx, elm_idx):
    from concourse.bass_utils import run_bass_kernel_spmd

    use_fp8 = True
    col_tile = True
    key = (use_fp8, col_tile)
    if key not in _CACHED:
        _CACHED[key] = build(use_fp8=use_fp8, col_tile=col_tile)
    nc = _CACHED[key]

    in_maps = make_in_maps(Alpha, K, p0, ref_idx, elm_idx, use_fp8=use_fp8)
    res = run_bass_kernel_spmd(nc, in_maps, core_ids=list(range(N_CORES)))
    return unshard(res.results, p0, elm_idx, col_tile=col_tile)


# revision 3
# speedup vs baseline: 2.1652x; 1.0103x over previous
"""Trainium2 Bass kernel for the AMASEQC scatter/matmul/gather problem.

Reference computation (P=32, E=4, R=8192, C=8192):
    Ag[p, e, r] = Alpha[p, ref_idx[e, r]]
    AK[p, e, c] = sum_r Ag[p, e, r] * K[e, r, c]
    pred[:, elm_idx[e, c]] = AK[:, e, c]
    out = pred + p0

Sharding (expert-style, 2 cores per element): core i handles element
e = i // 2 and column half h = i % 2 of K[e].

The gather/scatter permutations are pure data routing, so they are
folded into the host-side shard/unshard step (kernel() receives full
inputs and must slice them per core anyway).  Each core's device graph
is then a single streaming matmul at the HBM roofline:

  - Ag = Alpha[:, ref_idx[e]].T is pre-gathered on host, scaled by
    1/SCALE, tiled to [128, 64*32] bf16 (one dense 512 KB DMA),
  - K[e][:, half] is scaled by SCALE, cast to fp8e4 on host (TRN
    FP8_EXP4, max normal 240; |K*SCALE| < ~8), and tiled r-tile-major
    to [128, 64*4096] so the device streams it in 2 MB contiguous
    chunks on the two HWDGE queues (~33.5 MB/core, the roofline term),
  - 64 r-tiles x 8 n-chunks of matmul accumulate AK[param, c] into 2
    full PSUM banks, col-tiled 4-wide (tile_position=(0,32j)) so the
    M=32 matmuls run 4-at-a-time in the 128-wide PE array,
  - 2 full-width [128, 512] PSUM->SBUF drains + one dense 512 KB
    output DMA.  No indirect DMA anywhere on device.

Host unshards: AK chunks are reordered, scattered to columns
elm_idx[e, half], and p0 is added (cheap numpy ops on [32, 32768]).
"""

import sys

sys.path.insert(0, "/opt/trn_rl_repo")

import numpy as np

import concourse.bass as bass  # noqa: F401  (kept for parity with bass_utils)
import concourse.tile as tile
from concourse import bacc, mybir

P = 32
E = 4
R = 8192
C = 8192
N_CORES = 8
HALF_C = C // 2        # columns per core
N_RT = R // 128        # 64 r-tiles (contraction)
N_CH = HALF_C // 512   # 8 output chunks of 512
SCALE = 64.0           # K is stored as fp8(K*SCALE), Ag as bf16(Ag/SCALE)


def build(reps: int = 1, rt_per_dma: int = 1, kt_bufs: int = 14,
          col_tile: bool = True, use_fp8: bool = True, two_eng: bool = True):
    """Build the per-core Bass graph (pure streaming matmul form)."""
    assert N_RT % rt_per_dma == 0
    kdt = mybir.dt.float8e4 if use_fp8 else mybir.dt.bfloat16
    nc = bacc.Bacc("TRN2", debug=False, num_devices=N_CORES)
    kshard = nc.dram_tensor("kshard", [128, N_RT * HALF_C], kdt,
                            kind="ExternalInput")
    ag = nc.dram_tensor("ag", [128, N_RT * P], mybir.dt.bfloat16,
                        kind="ExternalInput")
    out_cols = 2 * 512 if col_tile else N_CH * 512
    out_rows = 128 if col_tile else P
    out = nc.dram_tensor("out", [out_rows, out_cols], mybir.dt.float32,
                         kind="ExternalOutput")

    with tile.TileContext(nc) as tc:
        with (
            tc.tile_pool(name="ag", bufs=2) as ag_pool,
            tc.tile_pool(name="kt", bufs=kt_bufs) as kt_pool,
            tc.tile_pool(name="stg", bufs=2) as stg_pool,
            tc.tile_pool(name="acc", bufs=1, space="PSUM") as acc_pool,
        ):
            for _ in range(reps):
                agt = ag_pool.tile([128, N_RT * P], mybir.dt.bfloat16)
                nc.scalar.dma_start(agt[:], ag.ap())

                if col_tile:
                    banks = [acc_pool.tile([128, 512], mybir.dt.float32,
                                           name=f"acc{b}", tag=f"acc{b}")
                             for b in range(2)]
                else:
                    banks = [acc_pool.tile([P, 512], mybir.dt.float32,
                                           name=f"acc{g}", tag=f"acc{g}")
                             for g in range(N_CH)]

                for rd in range(N_RT // rt_per_dma):
                    kt = kt_pool.tile([128, rt_per_dma * HALF_C], kdt)
                    eng = nc.scalar if (two_eng and rd % 2) else nc.sync
                    lo = rd * rt_per_dma * HALF_C
                    eng.dma_start(kt[:], kshard.ap()[:, lo:lo + rt_per_dma * HALF_C])
                    for sub in range(rt_per_dma):
                        rt = rd * rt_per_dma + sub
                        lhsT = agt[:, rt * P:(rt + 1) * P]
                        for g in range(N_CH):
                            rhs = kt[:, sub * HALF_C + g * 512:
                                     sub * HALF_C + (g + 1) * 512]
                            if col_tile:
                                j = g % 4
                                nc.tensor.matmul(
                                    banks[g // 4][j * 32:(j + 1) * 32, :],
                                    lhsT=lhsT, rhs=rhs,
                                    start=(rt == 0), stop=(rt == N_RT - 1),
                                    tile_position=(0, 32 * j),
                                )
                            else:
                                nc.tensor.matmul(
                                    banks[g][:], lhsT=lhsT, rhs=rhs,
                                    start=(rt == 0), stop=(rt == N_RT - 1),
                                )

                stage = stg_pool.tile([out_rows, out_cols], mybir.dt.float32)
                for b, bank in enumerate(banks):
                    nc.vector.tensor_copy(out=stage[:, b * 512:(b + 1) * 512],
                                          in_=bank[:])
                nc.sync.dma_start(out.ap(), stage[:])

    nc.compile()
    return nc


def make_in_maps(Alpha, K, p0, ref_idx, elm_idx, use_fp8: bool = True):
    """Host-side sharding: per-core pre-gathered Alpha and r-tile-major
    K shard, pre-scaled/cast so the device does no format work."""
    import ml_dtypes
    kdt = ml_dtypes.float8_e4m3 if use_fp8 else ml_dtypes.bfloat16
    Alpha = np.asarray(Alpha, np.float32)
    K = np.asarray(K, np.float32)
    ref_idx = np.asarray(ref_idx)
    in_maps = []
    for core in range(N_CORES):
        e, h = core // 2, core % 2
        agT = Alpha[:, ref_idx[e]].T                      # [8192, 32]
        if use_fp8:
            agT = agT * np.float32(1.0 / SCALE)
        agc = np.ascontiguousarray(
            agT.reshape(N_RT, 128, P).transpose(1, 0, 2)
        ).astype(ml_dtypes.bfloat16).reshape(128, N_RT * P)
        ksh = K[e, :, h * HALF_C:(h + 1) * HALF_C]        # [8192, 4096]
        if use_fp8:
            ksh = ksh * np.float32(SCALE)
        ksh = np.ascontiguousarray(ksh).astype(kdt)       # cast at 1 B/elem
        ksh = np.ascontiguousarray(
            ksh.reshape(N_RT, 128, HALF_C).transpose(1, 0, 2)
        ).reshape(128, N_RT * HALF_C)
        in_maps.append({"kshard": ksh, "ag": agc})
    return in_maps


_CACHED = {}


def unshard(results, p0, elm_idx, col_tile: bool = True):
    """Assemble the full [32, 32768] output from per-core dense results."""
    p0 = np.asarray(p0, np.float32)
    elm_idx = np.asarray(elm_idx)
    out = np.empty_like(p0)
    for core in range(N_CORES):
        e, h = core // 2, core % 2
        o = np.asarray(results[core]["out"], np.float32)
        if col_tile:
            # o[32*j + m, b*512 + c] = AK[m, (4*b + j)*512 + c]
            ak = o.reshape(4, 32, 2, 512).transpose(1, 2, 0, 3).reshape(P, HALF_C)
        else:
            ak = o
        cols = elm_idx[e, h * HALF_C:(h + 1) * HALF_C]
        out[:, cols] = ak + p0[:, cols]
    return out


def kernel(Alpha, K, p0, ref_idx, elm_idx):
    from concourse.bass_utils import run_bass_kernel_spmd

    use_fp8 = True
    col_tile = True
    key = (use_fp8, col_tile)
    if key not in _CACHED:
        _CACHED[key] = build(use_fp8=use_fp8, col_tile=col_tile)
    nc = _CACHED[key]

    in_maps = make_in_maps(Alpha, K, p0, ref_idx, elm_idx, use_fp8=use_fp8)
    res = run_bass_kernel_spmd(nc, in_maps, core_ids=list(range(N_CORES)))
    return unshard(res.results, p0, elm_idx, col_tile=col_tile)
